# revision 47
# baseline (speedup 1.0000x reference)
"""CharEmbeddingCNN Trainium2 kernel (fp8 DoubleRow, one-hot formulation).

Reference computation (per word of L=20 chars):
    xe = emb[x]                       # [L, 256] -> treated as [256, L]
    y_k = conv1d_valid(xe, w_k) + b_k # k in (3,4,5), 256 -> 256 channels
    out = relu(max over all (k, t) of y_k[:, t]) * (len != 0)

Strategy (data-parallel over 8 NeuronCores, 1024 words each):
  - Since xe columns are embedding rows, each conv tap is a table lookup:
    y_k[:, t] = sum_dk WE_k[dk][:, x[t+dk]] with WE_k[dk] = emb @ w_k[:,:,dk].T
    folded on host (float64) and quantized to fp8 e4m3 at a 2^8 scale.
    The rhs is then a one-hot encoding of the characters, which is EXACT in
    fp8, so the only numerical error is the single wta quantization
    (measured rel err ~1e-2 vs the 2e-2 budget).
  - The 256-deep contraction (one-hot over the alphabet) maps onto ONE
    fp8 matmul per (k, dk, oc) via perf_mode=DoubleRow (2 fp8 weights per
    PE cell): lhsT [128, 2, 128], rhs one-hot [128, 2, gw, lk], halving
    the matmul count vs bf16 and doubling the MAC rate.
  - Conv accumulates over dk in PSUM with shifted rhs windows. Word groups
    of 28 fill a PSUM bank (N = 504/476/448 f32).
  - Segment max over t via DVE reduce_max into per-k accumulators,
    incremental bias+max-combine (biases pre-scaled by 2^8), PE transpose
    to [word, ch], and a fused relu*(mask/2^8) on ScalarE on the way out,
    all overlapped with the matmul stream.
  - No gathers at all: the one-hot rhs (5.2 MB/core fp8) streams in from
    DRAM in word-chunks, trivially hidden behind compute.
"""

import numpy as np
import ml_dtypes
from contextlib import ExitStack

import concourse.bacc as bacc
import concourse.tile as tile
from concourse import mybir
from concourse.bass_utils import run_bass_kernel_spmd

F32 = mybir.dt.float32
BF16 = mybir.dt.bfloat16
FP8 = mybir.dt.float8e4
DR = mybir.MatmulPerfMode.DoubleRow

B, S, L = 64, 128, 20
EMB = 256
KS = (3, 4, 5)
NCORES = 8
W = (B * S) // NCORES          # words per core (1024)
NKDK = sum(KS)                 # 12 packed (k, dk) weight slices
GW = 28                        # words per matmul group (N = 504/476/448)
SCALE = 256.0                  # fp8 wta scale; undone by the output mask
WARMUP_MM = 7                  # dummy matmuls to bridge until the first
                               # group's input DMAs land (PE must stay busy
                               # continuously or the HAM clock-gate resets)
NP_FP8 = ml_dtypes.float8_e4m3  # TRN FP8_EXP4: bias 7, max normal 240


def _kdk_off(ki, dk):
    return sum(KS[:ki]) + dk


def build_bass(words=W, ng=37):
    """ng: number of 28-word groups actually computed. Words beyond ng*GW
    must be masked (lens==0): their rows are never written and stay zero in
    the preallocated output buffer."""
    nwb = words // 128          # output word-blocks of 128
    nw_e = min(ng * GW, words)  # words computed on device
    nc = bacc.Bacc(
        "TRN2",
        target_bir_lowering=False,
        debug=False,
        enable_asserts=False,
        num_swdge_queues=1,
    )

    ng_eff = -(-nw_e // GW)     # groups (28-word chunks) actually computed
    # one-hot, group-chunk-major: [p, group, chunk(c), word-in-group*L] so
    # every DMA chunk is one contiguous segment per partition
    oh_d = nc.dram_tensor("oh", [128, ng_eff * 2 * GW * L], FP8,
                          kind="ExternalInput").ap()
    wta_d = nc.dram_tensor("wta", [128, NKDK, 2, EMB], FP8,
                           kind="ExternalInput").ap()
    bias_d = nc.dram_tensor("bias", [128, 6], F32, kind="ExternalInput").ap()
    mask_d = nc.dram_tensor("maskp", [128, nwb], F32, kind="ExternalInput").ap()
    id_d = nc.dram_tensor("ident", [128, 128], BF16, kind="ExternalInput").ap()
    out_d = nc.dram_tensor("out", [words, EMB], F32, kind="ExternalOutput").ap()

    groups = []
    w0 = 0
    while w0 < nw_e:
        gw = min(GW, nw_e - w0)
        groups.append((w0, gw))
        w0 += gw
    # split the final group so the end-of-stream reduce+combine chain (the
    # unoverlapped tail) covers only a few words
    if groups and groups[-1][1] > 12:
        w0, gw = groups.pop()
        groups += [(w0, gw - 8), (w0 + gw - 8, 8)]

    with tile.TileContext(nc) as tc, ExitStack() as ctx:
        const_pool = ctx.enter_context(tc.tile_pool(name="const", bufs=1))
        oh_pool = ctx.enter_context(tc.tile_pool(name="ohp", bufs=1))
        # 8 PSUM banks: 3 for k3, 3 for k4, 2 for k5 accumulation; the
        # final-block transposes rotate through the k3 pool's bufs
        ps_pool = [
            ctx.enter_context(tc.tile_pool(name="psA", bufs=3, space="PSUM")),
            ctx.enter_context(tc.tile_pool(name="psB", bufs=3, space="PSUM")),
            ctx.enter_context(tc.tile_pool(name="psC", bufs=2, space="PSUM")),
        ]
        m_pool = ctx.enter_context(tc.tile_pool(name="m", bufs=1))
        tmp_pool = ctx.enter_context(tc.tile_pool(name="tmp", bufs=2))
        out_pool = ctx.enter_context(tc.tile_pool(name="outp", bufs=3))

        wta_t = const_pool.tile([128, NKDK, 2, EMB], FP8)
        oh_t = oh_pool.tile([128, ng_eff, 2, GW * L], FP8)
        oh_v = oh_d[:].rearrange("p (g c j) -> p g c j", g=ng_eff, c=2)

        # Input DMAs ordered so the first group's operands land first.
        nc.sync.dma_start(wta_t[:, 0:3], wta_d[:, 0:3])
        nc.sync.dma_start(oh_t[:, 0:1], oh_v[:, 0:1])
        nc.sync.dma_start(wta_t[:, 3:NKDK], wta_d[:, 3:NKDK])
        bias_t = const_pool.tile([128, 6], F32)
        nc.sync.dma_start(bias_t[:], bias_d[:])
        mask_t = const_pool.tile([128, nwb], F32)
        nc.sync.dma_start(mask_t[:], mask_d[:])
        ident = const_pool.tile([128, 128], BF16)
        nc.sync.dma_start(ident[:], id_d[:])
        # remaining one-hot chunks: small at first (stay ahead of the
        # matmul stream with minimal first-group latency), then coarse
        g0 = 1
        for ch_groups in (1, 2, 4, 8, 8, 8):
            g1 = min(g0 + ch_groups, ng_eff)
            if g1 > g0:
                nc.sync.dma_start(oh_t[:, g0:g1], oh_v[:, g0:g1])
            g0 = g1
        while g0 < ng_eff:
            g1 = min(g0 + 8, ng_eff)
            nc.sync.dma_start(oh_t[:, g0:g1], oh_v[:, g0:g1])
            g0 = g1

        M = {}
        for ki in range(3):
            for oc in range(2):
                M[(ki, oc)] = m_pool.tile(
                    [128, words], F32, tag=f"m{ki}{oc}", name=f"m{ki}{oc}")
        # final combined max in bf16 so the output transpose can go through
        # the DMA xbar (2-byte elements only) instead of stealing PE time
        C = [m_pool.tile([128, words], BF16, tag=f"c{oc}", name=f"c{oc}")
             for oc in range(2)]

        # PE warm-up on local scratch (no DMA dependency)
        scratch = const_pool.tile([128, 2, 256], FP8)
        nc.vector.memset(scratch[:], 0.0)
        warm = ps_pool[0].tile([128, 256], F32, tag="ps0")
        for _ in range(WARMUP_MM):
            nc.tensor.matmul(warm[:], scratch[:, :, 0:128], scratch[:],
                             start=True, stop=True, perf_mode=DR)

        wb_done = 0
        covered = 0

        def combine(hi):
            """Fold M into C for columns [covered, hi); bias is pre-scaled."""
            nonlocal covered
            sl = slice(covered, hi)
            n = hi - covered
            for oc in range(2):
                t4 = tmp_pool.tile([128, n], F32, tag="t4", name="t4")
                t5 = tmp_pool.tile([128, n], F32, tag="t5", name="t5")
                nc.scalar.add(
                    t4[:], M[(0, oc)][:, sl], bias_t[:, 3 * oc:3 * oc + 1])
                nc.scalar.add(
                    t5[:], M[(1, oc)][:, sl], bias_t[:, 3 * oc + 1:3 * oc + 2])
                nc.vector.tensor_max(t4[:], t4[:], t5[:])
                nc.scalar.add(
                    t5[:], M[(2, oc)][:, sl], bias_t[:, 3 * oc + 2:3 * oc + 3])
                nc.vector.tensor_max(C[oc][:, sl], t4[:], t5[:])
            covered = hi

        def emit_ready():
            """Emit finished 128-word output blocks: transpose to [word, ch]
            via the DMA xbar mid-stream (off the PE; ~1.2us each but the DMA
            queue is idle), or on the PE for the final block (PE is idle by
            then and its transpose is ~3x faster, shortening the tail), then
            a fused relu*(mask/SCALE) on ScalarE."""
            nonlocal wb_done
            while (wb_done + 1) * 128 <= covered:
                wb = wb_done
                last = (wb + 1) * 128 == nw_e
                ot = out_pool.tile([128, 2, 128], F32, tag="ot", name="ot")
                for oc in range(2):
                    if last:
                        pst = ps_pool[0].tile([128, 128], BF16, tag="ps0",
                                              name="pst")
                        nc.tensor.transpose(
                            pst[:], C[oc][:, wb * 128:(wb + 1) * 128],
                            ident[:])
                        tt = pst
                    else:
                        tt = out_pool.tile([128, 128], BF16, tag="tt",
                                           name="tt")
                        nc.sync.dma_start_transpose(
                            tt[:], C[oc][:, wb * 128:(wb + 1) * 128])
                    nc.scalar.activation(
                        ot[:, oc, :], tt[:], mybir.ActivationFunctionType.Relu,
                        scale=mask_t[:, wb:wb + 1])
                    if last:
                        # per-oc DMA so oc0's output streams out while oc1's
                        # relu is still running (shortens the tail)
                        nc.sync.dma_start(
                            out_d[wb * 128:(wb + 1) * 128,
                                  oc * 128:(oc + 1) * 128], ot[:, oc, :])
                if not last:
                    nc.sync.dma_start(
                        out_d[wb * 128:(wb + 1) * 128, :],
                        ot[:].rearrange("p c j -> p (c j)"))
                wb_done += 1

        for (w0, gw) in groups:
            emit_ready()
            ci, wo = divmod(w0, GW)
            gv = oh_t[:, ci, :, :].rearrange("p c (w t) -> p c w t", t=L)
            for oc in range(2):
                for ki, k in enumerate(KS):
                    lk = L - k + 1
                    # the rhs pair-fetch rounds an odd innermost extent up
                    # to even (lk=17 streams like 18); transpose k4's free
                    # dims so the inner extent is the (even) word count
                    tr = lk % 2 == 1
                    shape = [128, lk, gw] if tr else [128, gw, lk]
                    ps = ps_pool[ki].tile(shape, F32,
                                          tag=f"ps{ki}", name=f"ps{ki}")
                    for dk in range(k):
                        rhs = gv[:, :, wo:wo + gw, dk:dk + lk]
                        if tr:
                            rhs = rhs.rearrange("p c w t -> p c t w")
                        nc.tensor.matmul(
                            ps[:],
                            wta_t[:, _kdk_off(ki, dk), :,
                                  oc * 128:(oc + 1) * 128],
                            rhs,
                            start=(dk == 0), stop=(dk == k - 1),
                            perf_mode=DR,
                        )
                    red_in = (ps[:].rearrange("p t w -> p w t")
                              if tr else ps[:])
                    nc.vector.reduce_max(
                        M[(ki, oc)][:, w0:w0 + gw], red_in,
                        axis=mybir.AxisListType.X)
            # combine every ~128 words; per-group in the last block so the
            # final emit chain after the last matmul is as short as possible
            if w0 + gw - covered >= 128 or w0 + gw > nw_e - 128:
                combine(w0 + gw)
        emit_ready()
        assert covered == nw_e
        pw = nw_e - wb_done * 128
        if pw > 0:
            # partial final block (mask-packed inputs: the remaining words
            # of this block and all later blocks are lens==0 -> output rows
            # stay zero in the preallocated buffer)
            wb = wb_done
            ot = out_pool.tile([128, 2, 128], F32, tag="ot", name="ot")
            for oc in range(2):
                pst = ps_pool[0].tile([128, 128], BF16, tag="ps0",
                                      name="pst")
                nc.tensor.transpose(
                    pst[0:pw, :], C[oc][:, wb * 128:wb * 128 + pw], ident[:])
                nc.scalar.activation(
                    ot[0:pw, oc, :], pst[0:pw, :],
                    mybir.ActivationFunctionType.Relu,
                    scale=mask_t[0:pw, wb:wb + 1])
                nc.sync.dma_start(
                    out_d[wb * 128:wb * 128 + pw,
                          oc * 128:(oc + 1) * 128], ot[0:pw, oc, :])

    nc.compile()
    return nc


def prep_shared(emb, w3, w4, w5, b3, b4, b5):
    emb64 = np.asarray(emb, np.float64)
    wta = np.empty((128, NKDK, 2, EMB), dtype=NP_FP8)
    for ki, w in enumerate((w3, w4, w5)):
        k = KS[ki]
        w64 = np.asarray(w, np.float64)
        for dk in range(k):
            # t[a, o] = sum_i emb[a, i] w[o, i, dk], scaled into fp8 range
            t = (emb64 @ w64[:, :, dk].T) * SCALE
            wta[:, _kdk_off(ki, dk)] = (
                t.reshape(2, 128, EMB).transpose(1, 0, 2).astype(NP_FP8))
    bias = np.empty((128, 6), dtype=np.float32)
    for oc in range(2):
        for ki, b in enumerate((b3, b4, b5)):
            bias[:, 3 * oc + ki] = (
                np.asarray(b, np.float64)[oc * 128:(oc + 1) * 128] * SCALE)
    ident = np.eye(128, dtype=ml_dtypes.bfloat16)
    return wta, bias, ident


def prep_core(xf, lensf, ng, words=W):
    """Per-core one-hot + mask packing. xf: [words, L] int32, lensf: [words].
    One-hot is group-chunk-major: oh[p, g, c, w*L+t] = (xf[28g+w, t] ==
    c*128 + p), fp8 exact. Only the first ng*GW words are encoded (the rest
    are masked and never computed)."""
    nw_e = min(ng * GW, words)
    ng_eff = -(-nw_e // GW)
    xe = xf[:nw_e]
    pad = ng_eff * GW - nw_e
    if pad:
        xe = np.concatenate([xe, np.zeros((pad, L), xe.dtype)])
    pos = xe.reshape(-1)
    onehot = (np.arange(EMB, dtype=np.int32)[:, None] == pos[None, :])
    oh = (onehot.reshape(2, 128, ng_eff, GW * L).transpose(1, 2, 0, 3)
          .reshape(128, -1).astype(NP_FP8))
    nwb = words // 128
    maskp = ((lensf.reshape(nwb, 128).T != 0).astype(np.float32)
             * np.float32(1.0 / SCALE))
    return np.ascontiguousarray(oh), np.ascontiguousarray(maskp)


_CACHE = {}


def _get_nc(words=W, ng=37):
    if (words, ng) not in _CACHE:
        _CACHE[(words, ng)] = build_bass(words, ng)
    return _CACHE[(words, ng)]


def shard_words(lensf):
    """Balance unmasked (lens!=0) words across cores, masked words last.
    Words past the computed region produce zero rows for free, so each core
    only needs ceil(max_unmasked_per_core / GW) matmul groups."""
    mask = lensf != 0
    unm = np.flatnonzero(mask)
    msk = np.flatnonzero(~mask)
    parts = np.array_split(unm, NCORES)
    idx = []
    mi = 0
    for c in range(NCORES):
        need = W - len(parts[c])
        idx.append(np.concatenate([parts[c], msk[mi:mi + need]]))
        mi += need
    max_unm = max(len(p) for p in parts)
    ng = min(-(-max_unm // GW), -(-W // GW))
    return np.stack(idx), ng


def run(x, lens, emb, w3, b3, w4, b4, w5, b5, trace=False, **spmd_kwargs):
    x = np.asarray(x)
    lens = np.asarray(lens)
    wta, bias, ident = prep_shared(
        np.asarray(emb), np.asarray(w3), np.asarray(w4), np.asarray(w5),
        np.asarray(b3), np.asarray(b4), np.asarray(b5))
    xf = x.reshape(B * S, L)
    lensf = lens.reshape(B * S)
    idx, ng = shard_words(lensf)
    nc = _get_nc(W, ng)
    in_maps = []
    for c in range(NCORES):
        rows = idx[c]
        oh, maskp = prep_core(xf[rows], lensf[rows], ng)
        in_maps.append({
            "oh": oh, "wta": wta, "bias": bias, "maskp": maskp,
            "ident": ident,
        })
    res = run_bass_kernel_spmd(
        nc, in_maps, core_ids=list(range(NCORES)), trace=trace, **spmd_kwargs)
    out = np.concatenate([r["out"] for r in res.results], axis=0)
    full = np.empty((B * S, EMB), dtype=np.float32)
    full[idx.reshape(-1)] = out
    return np.ascontiguousarray(full.reshape(B, S, EMB)), res


def kernel(x, lens, emb, w3, b3, w4, b4, w5, b5, **unused):
    out, _ = run(x, lens, emb, w3, b3, w4, b4, w5, b5)
    return out


# revision 50
# speedup vs baseline: 2.1811x; 2.1811x over previous
"""CharEmbeddingCNN Trainium2 kernel (fp8 DoubleRow, one-hot formulation).

Reference computation (per word of L=20 chars):
    xe = emb[x]                       # [L, 256] -> treated as [256, L]
    y_k = conv1d_valid(xe, w_k) + b_k # k in (3,4,5), 256 -> 256 channels
    out = relu(max over all (k, t) of y_k[:, t]) * (len != 0)

Strategy (data-parallel over 8 NeuronCores, 1024 words each):
  - Since xe columns are embedding rows, each conv tap is a table lookup:
    y_k[:, t] = sum_dk WE_k[dk][:, x[t+dk]] with WE_k[dk] = emb @ w_k[:,:,dk].T
    folded on host (float64) and quantized to fp8 e4m3 at a 2^8 scale.
    The rhs is then a one-hot encoding of the characters, which is EXACT in
    fp8, so the only numerical error is the single wta quantization
    (measured rel err ~1e-2 vs the 2e-2 budget).
  - The 256-deep contraction (one-hot over the alphabet) maps onto ONE
    fp8 matmul per (k, dk, oc) via perf_mode=DoubleRow (2 fp8 weights per
    PE cell): lhsT [128, 2, 128], rhs one-hot [128, 2, gw, lk], halving
    the matmul count vs bf16 and doubling the MAC rate.
  - Conv accumulates over dk in PSUM with shifted rhs windows. Word groups
    of 28 fill a PSUM bank (N = 504/476/448 f32).
  - Segment max over t via DVE reduce_max into per-k accumulators,
    incremental bias+max-combine (biases pre-scaled by 2^8), PE transpose
    to [word, ch], and a fused relu*(mask/2^8) on ScalarE on the way out,
    all overlapped with the matmul stream.
  - No gathers at all: the one-hot rhs (5.2 MB/core fp8) streams in from
    DRAM in word-chunks, trivially hidden behind compute.
"""

import numpy as np
import ml_dtypes
from contextlib import ExitStack

import concourse.bacc as bacc
import concourse.tile as tile
from concourse import mybir
from concourse.bass_utils import run_bass_kernel_spmd

F32 = mybir.dt.float32
BF16 = mybir.dt.bfloat16
FP8 = mybir.dt.float8e4
DR = mybir.MatmulPerfMode.DoubleRow

B, S, L = 64, 128, 20
EMB = 256
KS = (3, 4, 5)
NCORES = 8
W = (B * S) // NCORES          # words per core (1024)
NKDK = sum(KS)                 # 12 packed (k, dk) weight slices
GW = 28                        # words per matmul group (N = 504/476/448)
SCALE = 256.0                  # fp8 wta scale; undone by the output mask
WARMUP_MM = 8                  # dummy matmuls to bridge until the first
                               # group's input DMAs land (PE must stay busy
                               # continuously or the HAM clock-gate resets)
NP_FP8 = ml_dtypes.float8_e4m3  # TRN FP8_EXP4: bias 7, max normal 240


def _kdk_off(ki, dk):
    return sum(KS[:ki]) + dk


def build_bass(words=W, ng=37):
    """ng: number of 28-word groups actually computed. Words beyond ng*GW
    must be masked (lens==0): their rows are never written and stay zero in
    the preallocated output buffer."""
    nwb = words // 128          # output word-blocks of 128
    nw_e = min(ng * GW, words)  # words computed on device
    nc = bacc.Bacc(
        "TRN2",
        target_bir_lowering=False,
        debug=False,
        enable_asserts=False,
        num_swdge_queues=1,
    )

    ng_eff = -(-nw_e // GW)     # groups (28-word chunks) actually computed
    # one-hot, group-chunk-major: [p, group, chunk(c), word-in-group*L] so
    # every DMA chunk is one contiguous segment per partition
    oh_d = nc.dram_tensor("oh", [128, ng_eff * 2 * GW * L], FP8,
                          kind="ExternalInput").ap()
    wta_d = nc.dram_tensor("wta", [128, NKDK, 2, EMB], FP8,
                           kind="ExternalInput").ap()
    bias_d = nc.dram_tensor("bias", [128, 6], F32, kind="ExternalInput").ap()
    mask_d = nc.dram_tensor("maskp", [128, nwb], F32, kind="ExternalInput").ap()
    id_d = nc.dram_tensor("ident", [128, 128], BF16, kind="ExternalInput").ap()
    out_d = nc.dram_tensor("out", [words, EMB], F32, kind="ExternalOutput").ap()

    groups = []
    w0 = 0
    while w0 < nw_e:
        gw = min(GW, nw_e - w0)
        groups.append((w0, gw))
        w0 += gw
    # split the final group so the end-of-stream reduce+combine chain (the
    # unoverlapped tail) covers only a few words
    if groups and groups[-1][1] > 12:
        w0, gw = groups.pop()
        groups += [(w0, gw - 8), (w0 + gw - 8, 8)]

    with tile.TileContext(nc) as tc, ExitStack() as ctx:
        const_pool = ctx.enter_context(tc.tile_pool(name="const", bufs=1))
        oh_pool = ctx.enter_context(tc.tile_pool(name="ohp", bufs=1))
        # 8 PSUM banks: 3 for k3, 3 for k4, 2 for k5 accumulation; the
        # final-block transposes rotate through the k3 pool's bufs
        ps_pool = [
            ctx.enter_context(tc.tile_pool(name="psA", bufs=3, space="PSUM")),
            ctx.enter_context(tc.tile_pool(name="psB", bufs=3, space="PSUM")),
            ctx.enter_context(tc.tile_pool(name="psC", bufs=2, space="PSUM")),
        ]
        m_pool = ctx.enter_context(tc.tile_pool(name="m", bufs=1))
        tmp_pool = ctx.enter_context(tc.tile_pool(name="tmp", bufs=2))
        out_pool = ctx.enter_context(tc.tile_pool(name="outp", bufs=3))

        wta_t = const_pool.tile([128, NKDK, 2, EMB], FP8)
        oh_t = oh_pool.tile([128, ng_eff, 2, GW * L], FP8)
        oh_v = oh_d[:].rearrange("p (g c j) -> p g c j", g=ng_eff, c=2)

        # Input DMAs ordered so the first group's operands land first.
        nc.sync.dma_start(wta_t[:, 0:3], wta_d[:, 0:3])
        nc.sync.dma_start(oh_t[:, 0:1], oh_v[:, 0:1])
        nc.sync.dma_start(wta_t[:, 3:NKDK], wta_d[:, 3:NKDK])
        bias_t = const_pool.tile([128, 6], F32)
        nc.sync.dma_start(bias_t[:], bias_d[:])
        mask_t = const_pool.tile([128, nwb], F32)
        nc.sync.dma_start(mask_t[:], mask_d[:])
        ident = const_pool.tile([128, 128], BF16)
        nc.sync.dma_start(ident[:], id_d[:])
        # remaining one-hot chunks: small at first (stay ahead of the
        # matmul stream with minimal first-group latency), then coarse
        g0 = 1
        for ch_groups in (1, 2, 4, 8, 8, 8):
            g1 = min(g0 + ch_groups, ng_eff)
            if g1 > g0:
                nc.sync.dma_start(oh_t[:, g0:g1], oh_v[:, g0:g1])
            g0 = g1
        while g0 < ng_eff:
            g1 = min(g0 + 8, ng_eff)
            nc.sync.dma_start(oh_t[:, g0:g1], oh_v[:, g0:g1])
            g0 = g1

        M = {}
        for ki in range(3):
            for oc in range(2):
                M[(ki, oc)] = m_pool.tile(
                    [128, words], F32, tag=f"m{ki}{oc}", name=f"m{ki}{oc}")
        # final combined max in bf16 so the output transpose can go through
        # the DMA xbar (2-byte elements only) instead of stealing PE time
        C = [m_pool.tile([128, words], BF16, tag=f"c{oc}", name=f"c{oc}")
             for oc in range(2)]

        # PE warm-up on local scratch (no DMA dependency). Each warmup gets
        # a fresh rotating psum tile: reusing one tile makes every matmul
        # wait on the previous one's completion semaphore (measured ~1us of
        # mid-warmup gaps, which resets the HAM busy window).
        scratch = const_pool.tile([128, 2, 256], FP8)
        nc.vector.memset(scratch[:], 0.0)
        for _ in range(WARMUP_MM):
            warm = ps_pool[0].tile([128, 256], F32, tag="ps0")
            nc.tensor.matmul(warm[:], scratch[:, :, 0:128], scratch[:],
                             start=True, stop=True, perf_mode=DR)

        wb_done = 0
        covered = 0

        def combine(hi):
            """Fold M into C for columns [covered, hi); bias is pre-scaled."""
            nonlocal covered
            sl = slice(covered, hi)
            n = hi - covered
            for oc in range(2):
                t4 = tmp_pool.tile([128, n], F32, tag="t4", name="t4")
                t5 = tmp_pool.tile([128, n], F32, tag="t5", name="t5")
                nc.scalar.add(
                    t4[:], M[(0, oc)][:, sl], bias_t[:, 3 * oc:3 * oc + 1])
                nc.scalar.add(
                    t5[:], M[(1, oc)][:, sl], bias_t[:, 3 * oc + 1:3 * oc + 2])
                nc.vector.tensor_max(t4[:], t4[:], t5[:])
                nc.scalar.add(
                    t5[:], M[(2, oc)][:, sl], bias_t[:, 3 * oc + 2:3 * oc + 3])
                nc.vector.tensor_max(C[oc][:, sl], t4[:], t5[:])
            covered = hi

        def emit_ready():
            """Emit finished 128-word output blocks: transpose to [word, ch]
            via the DMA xbar mid-stream (off the PE; ~1.2us each but the DMA
            queue is idle), or on the PE for the final block (PE is idle by
            then and its transpose is ~3x faster, shortening the tail), then
            a fused relu*(mask/SCALE) on ScalarE."""
            nonlocal wb_done
            while (wb_done + 1) * 128 <= covered:
                wb = wb_done
                last = (wb + 1) * 128 == nw_e
                ot = out_pool.tile([128, 2, 128], F32, tag="ot", name="ot")
                for oc in range(2):
                    if last:
                        pst = ps_pool[0].tile([128, 128], BF16, tag="ps0",
                                              name="pst")
                        nc.tensor.transpose(
                            pst[:], C[oc][:, wb * 128:(wb + 1) * 128],
                            ident[:])
                        tt = pst
                    else:
                        tt = out_pool.tile([128, 128], BF16, tag="tt",
                                           name="tt")
                        nc.sync.dma_start_transpose(
                            tt[:], C[oc][:, wb * 128:(wb + 1) * 128])
                    nc.scalar.activation(
                        ot[:, oc, :], tt[:], mybir.ActivationFunctionType.Relu,
                        scale=mask_t[:, wb:wb + 1])
                    if last:
                        # per-oc DMA so oc0's output streams out while oc1's
                        # relu is still running (shortens the tail)
                        nc.sync.dma_start(
                            out_d[wb * 128:(wb + 1) * 128,
                                  oc * 128:(oc + 1) * 128], ot[:, oc, :])
                if not last:
                    nc.sync.dma_start(
                        out_d[wb * 128:(wb + 1) * 128, :],
                        ot[:].rearrange("p c j -> p (c j)"))
                wb_done += 1

        for (w0, gw) in groups:
            emit_ready()
            ci, wo = divmod(w0, GW)
            gv = oh_t[:, ci, :, :].rearrange("p c (w t) -> p c w t", t=L)
            for oc in range(2):
                for ki, k in enumerate(KS):
                    lk = L - k + 1
                    ps = ps_pool[ki].tile([128, gw, lk], F32,
                                          tag=f"ps{ki}", name=f"ps{ki}")
                    for dk in range(k):
                        nc.tensor.matmul(
                            ps[:],
                            wta_t[:, _kdk_off(ki, dk), :,
                                  oc * 128:(oc + 1) * 128],
                            gv[:, :, wo:wo + gw, dk:dk + lk],
                            start=(dk == 0), stop=(dk == k - 1),
                            perf_mode=DR,
                        )
                    nc.vector.reduce_max(
                        M[(ki, oc)][:, w0:w0 + gw], ps[:],
                        axis=mybir.AxisListType.X)
            # combine every ~128 words; per-group in the last block so the
            # final emit chain after the last matmul is as short as possible
            if w0 + gw - covered >= 128 or w0 + gw > nw_e - 128:
                combine(w0 + gw)
        emit_ready()
        assert covered == nw_e
        pw = nw_e - wb_done * 128
        if pw > 0:
            # partial final block (mask-packed inputs: the remaining words
            # of this block and all later blocks are lens==0 -> output rows
            # stay zero in the preallocated buffer)
            wb = wb_done
            ot = out_pool.tile([128, 2, 128], F32, tag="ot", name="ot")
            for oc in range(2):
                pst = ps_pool[0].tile([128, 128], BF16, tag="ps0",
                                      name="pst")
                nc.tensor.transpose(
                    pst[0:pw, :], C[oc][:, wb * 128:wb * 128 + pw], ident[:])
                nc.scalar.activation(
                    ot[0:pw, oc, :], pst[0:pw, :],
                    mybir.ActivationFunctionType.Relu,
                    scale=mask_t[0:pw, wb:wb + 1])
                nc.sync.dma_start(
                    out_d[wb * 128:wb * 128 + pw,
                          oc * 128:(oc + 1) * 128], ot[0:pw, oc, :])

    nc.compile()
    return nc


def prep_shared(emb, w3, w4, w5, b3, b4, b5):
    emb64 = np.asarray(emb, np.float64)
    wta = np.empty((128, NKDK, 2, EMB), dtype=NP_FP8)
    for ki, w in enumerate((w3, w4, w5)):
        k = KS[ki]
        w64 = np.asarray(w, np.float64)
        for dk in range(k):
            # t[a, o] = sum_i emb[a, i] w[o, i, dk], scaled into fp8 range
            t = (emb64 @ w64[:, :, dk].T) * SCALE
            wta[:, _kdk_off(ki, dk)] = (
                t.reshape(2, 128, EMB).transpose(1, 0, 2).astype(NP_FP8))
    bias = np.empty((128, 6), dtype=np.float32)
    for oc in range(2):
        for ki, b in enumerate((b3, b4, b5)):
            bias[:, 3 * oc + ki] = (
                np.asarray(b, np.float64)[oc * 128:(oc + 1) * 128] * SCALE)
    ident = np.eye(128, dtype=ml_dtypes.bfloat16)
    return wta, bias, ident


def prep_core(xf, lensf, ng, words=W):
    """Per-core one-hot + mask packing. xf: [words, L] int32, lensf: [words].
    One-hot is group-chunk-major: oh[p, g, c, w*L+t] = (xf[28g+w, t] ==
    c*128 + p), fp8 exact. Only the first ng*GW words are encoded (the rest
    are masked and never computed)."""
    nw_e = min(ng * GW, words)
    ng_eff = -(-nw_e // GW)
    xe = xf[:nw_e]
    pad = ng_eff * GW - nw_e
    if pad:
        xe = np.concatenate([xe, np.zeros((pad, L), xe.dtype)])
    pos = xe.reshape(-1)
    onehot = (np.arange(EMB, dtype=np.int32)[:, None] == pos[None, :])
    oh = (onehot.reshape(2, 128, ng_eff, GW * L).transpose(1, 2, 0, 3)
          .reshape(128, -1).astype(NP_FP8))
    nwb = words // 128
    maskp = ((lensf.reshape(nwb, 128).T != 0).astype(np.float32)
             * np.float32(1.0 / SCALE))
    return np.ascontiguousarray(oh), np.ascontiguousarray(maskp)


_CACHE = {}


def _get_nc(words=W, ng=37):
    if (words, ng) not in _CACHE:
        _CACHE[(words, ng)] = build_bass(words, ng)
    return _CACHE[(words, ng)]


def shard_words(lensf):
    """Balance unmasked (lens!=0) words across cores, masked words last.
    Words past the computed region produce zero rows for free, so each core
    only needs ceil(max_unmasked_per_core / GW) matmul groups."""
    mask = lensf != 0
    unm = np.flatnonzero(mask)
    msk = np.flatnonzero(~mask)
    parts = np.array_split(unm, NCORES)
    idx = []
    mi = 0
    for c in range(NCORES):
        need = W - len(parts[c])
        idx.append(np.concatenate([parts[c], msk[mi:mi + need]]))
        mi += need
    max_unm = max(len(p) for p in parts)
    ng = min(-(-max_unm // GW), -(-W // GW))
    return np.stack(idx), ng


def run(x, lens, emb, w3, b3, w4, b4, w5, b5, trace=False, **spmd_kwargs):
    x = np.asarray(x)
    lens = np.asarray(lens)
    wta, bias, ident = prep_shared(
        np.asarray(emb), np.asarray(w3), np.asarray(w4), np.asarray(w5),
        np.asarray(b3), np.asarray(b4), np.asarray(b5))
    xf = x.reshape(B * S, L)
    lensf = lens.reshape(B * S)
    idx, ng = shard_words(lensf)
    nc = _get_nc(W, ng)
    in_maps = []
    for c in range(NCORES):
        rows = idx[c]
        oh, maskp = prep_core(xf[rows], lensf[rows], ng)
        in_maps.append({
            "oh": oh, "wta": wta, "bias": bias, "maskp": maskp,
            "ident": ident,
        })
    res = run_bass_kernel_spmd(
        nc, in_maps, core_ids=list(range(NCORES)), trace=trace, **spmd_kwargs)
    out = np.concatenate([r["out"] for r in res.results], axis=0)
    full = np.empty((B * S, EMB), dtype=np.float32)
    full[idx.reshape(-1)] = out
    return np.ascontiguousarray(full.reshape(B, S, EMB)), res


def kernel(x, lens, emb, w3, b3, w4, b4, w5, b5, **unused):
    out, _ = run(x, lens, emb, w3, b3, w4, b4, w5, b5)
    return out
